# revision 5
# baseline (speedup 1.0000x reference)
"""Masked attention kernel v3 for Trainium2, 8 NeuronCores.

Measured platform behavior: per-instruction cost is dominated by instruction
FETCH of the unrolled stream (~49us/matmul, ~143us/ACT op when fully
unrolled); the same dynamic work inside a small For_i hardware-loop body runs
~10x faster (IRAM-resident). v3 therefore wraps the per-head program in
nested hardware loops (niter x heads) with a compact static body:
  static: 64 score MMs + 16 mask-adds (DVE) + 16 exps (ACT) + 64 PV MMs
          + 1 evac + 4 DMAs  (~170 instrs, <=256 per engine -> IRAM-resident)
  dynamic head index appears only in the 4 DRAM DMA access patterns (ds()).
Algorithm (per head, all transposed, softmax over the partition axis):
  scoresT = K^T.T @ Q^T (f32r, PSUM [128,2048] per t-block)
  scoresT += maskbias (bf16 -1e9 additive, DVE, in-PSUM)
  probsT = exp(scoresT/8) -> SBUF bf16 (no max-subtraction; |s|/8 <= ~6)
  outT[65,2048] = [V|1]^T @ probsT accumulated bf16xbf16->fp32 over t-blocks;
  row 64 = softmax denominator. Normalization happens on HOST after gather.
Sharding: core c = batch c//2, heads (c%2)*8..+8.
"""

import numpy as np
import ml_dtypes

import concourse.tile as tile
from concourse import bacc, mybir
from concourse.bass import ds
from concourse.bass_utils import run_bass_kernel_spmd

B, H, S, D = 4, 16, 2048, 64
N_CORES = 8
HPC = (B * H) // N_CORES  # heads per core

_NC_CACHE = {}


def build_nc(hpc=HPC, n_tb=S // 128, sq=S, niter=1):
    st = n_tb * 128
    hw = 512
    f32, f32r, bf16 = mybir.dt.float32, mybir.dt.float32r, mybir.dt.bfloat16
    EXP = mybir.ActivationFunctionType.Exp

    nc = bacc.Bacc("TRN2", target_bir_lowering=False, debug=False,
                   num_devices=N_CORES)
    QT = nc.dram_tensor("QT", [hpc * D, sq], f32r, kind="ExternalInput")
    KT = nc.dram_tensor("KT", [hpc * D, st], f32r, kind="ExternalInput")
    VE = nc.dram_tensor("VE", [hpc * 128, n_tb * (D + 1)], bf16,
                        kind="ExternalInput")
    KPB = nc.dram_tensor("KPB", [128, n_tb, sq], bf16, kind="ExternalInput")
    OD = nc.dram_tensor("OD", [hpc * (D + 1), sq], f32, kind="ExternalOutput")

    with tile.TileContext(nc) as tc:
        with (
            tc.tile_pool(name="mask", bufs=1) as maskp,
            tc.tile_pool(name="kqv", bufs=1) as kqvp,
            tc.tile_pool(name="pr", bufs=1) as prp,
            tc.tile_pool(name="fin", bufs=1) as finp,
            tc.tile_pool(name="sc", bufs=1, space="PSUM") as scp,
            tc.tile_pool(name="oa", bufs=1, space="PSUM") as oaccp,
        ):
            kpb = maskp.tile([128, n_tb, sq], bf16)
            nc.sync.dma_start(out=kpb[:], in_=KPB.ap())
            kt = kqvp.tile([D, st], f32r)
            qt = kqvp.tile([D, sq], f32r)
            ve = kqvp.tile([128, n_tb, D + 1], bf16)
            ktB = kqvp.tile([D, st], f32r)
            qtB = kqvp.tile([D, sq], f32r)
            veB = kqvp.tile([128, n_tb, D + 1], bf16)
            pr = prp.tile([128, n_tb, sq], bf16)
            sc0 = scp.tile([128, 1024], f32)
            sc1 = scp.tile([128, 1024], f32)
            sc_tiles = [sc0, sc1]
            oacc = oaccp.tile([D + 1, sq], f32)
            of = finp.tile([D + 1, sq], f32)
            ofB = finp.tile([D + 1, sq], f32)
            def head_body(hh, kt, qt, ve, of):
                nc.sync.dma_start(out=kt[:], in_=KT.ap()[ds(hh * D, D)])
                nc.sync.dma_start(out=qt[:], in_=QT.ap()[ds(hh * D, D)])
                nc.sync.dma_start(out=ve[:], in_=VE.ap()[ds(hh * 128, 128)])
                for tb in range(n_tb):
                    t0 = tb * 128
                    for half in range(2):
                        s0 = half * 1024
                        sc = sc_tiles[(2 * tb + half) % 2]
                        for w in range(2):
                            o = w * hw
                            nc.tensor.matmul(sc[:, o:o + hw],
                                             kt[:, t0:t0 + 128],
                                             qt[:, s0 + o:s0 + o + hw],
                                             start=True, stop=True)
                        nc.vector.tensor_add(sc[:], sc[:],
                                             kpb[:, tb, s0:s0 + 1024])
                        nc.scalar.activation(
                            out=pr[:, tb, s0:s0 + 1024], in_=sc[:],
                            func=EXP, scale=0.125)
                for tb in range(n_tb):
                    for w in range(sq // hw):
                        nc.tensor.matmul(oacc[:, w * hw:(w + 1) * hw],
                                         ve[:, tb, :],
                                         pr[:, tb, w * hw:(w + 1) * hw],
                                         start=(tb == 0),
                                         stop=(tb == n_tb - 1))
                nc.vector.tensor_copy(out=of[:], in_=oacc[:])
                nc.sync.dma_start(out=OD.ap()[ds(hh * (D + 1), D + 1)],
                                  in_=of[:])

            with tc.For_i(0, niter, 1, name="outer") as _:
                with tc.For_i(0, hpc // 2, 1, name="heads",
                              staggered_reset=True) as g:
                    head_body(g * 2, kt, qt, ve, of)
                    head_body(g * 2 + 1, ktB, qtB, veB, ofB)
    nc.compile()
    return nc


def _get_nc(**kw):
    key = tuple(sorted(kw.items()))
    if key not in _NC_CACHE:
        _NC_CACHE[key] = build_nc(**kw)
    return _NC_CACHE[key]


def make_in_maps(Q, K, V, mask):
    bf16 = ml_dtypes.bfloat16
    QTf = np.ascontiguousarray(Q.transpose(0, 1, 3, 2), dtype=np.float32)
    KTf = np.ascontiguousarray(K.transpose(0, 1, 3, 2), dtype=np.float32)
    ones = np.ones((B, H, S, 1), np.float32)
    VEf = np.concatenate([np.asarray(V, np.float32), ones], axis=-1)
    # [B,H,S,65] -> [B,H,16,128,65] -> [B,H,128,16,65]
    VEf = VEf.reshape(B, H, S // 128, 128, D + 1).transpose(0, 1, 3, 2, 4)
    VEf = np.ascontiguousarray(VEf).astype(bf16)
    # KPB[b, p, tb, s] = -1e9 where mask[b,0,s,tb*128+p] else 0
    mT = np.asarray(mask[:, 0]).transpose(0, 2, 1)          # [B, t, s]
    KPBf = np.where(mT, np.float32(-1e9), np.float32(0)).astype(bf16)
    KPBf = KPBf.reshape(B, S // 128, 128, S).transpose(0, 2, 1, 3)
    KPBf = np.ascontiguousarray(KPBf)                        # [B,128,16,S]
    in_maps = []
    for c in range(N_CORES):
        b, h0 = c // 2, (c % 2) * HPC
        in_maps.append({
            "QT": np.ascontiguousarray(QTf[b, h0:h0 + HPC]).reshape(
                HPC * D, S),
            "KT": np.ascontiguousarray(KTf[b, h0:h0 + HPC]).reshape(
                HPC * D, S),
            "VE": np.ascontiguousarray(VEf[b, h0:h0 + HPC]).reshape(
                HPC * 128, (S // 128) * (D + 1)),
            "KPB": KPBf[b],
        })
    return in_maps


def kernel(Q, K, V, mask):
    nc = _get_nc()
    in_maps = make_in_maps(Q, K, V, mask)
    res = run_bass_kernel_spmd(nc, in_maps, core_ids=list(range(N_CORES)))
    out = np.empty((B, H, S, D), np.float32)
    for c in range(N_CORES):
        b, h0 = c // 2, (c % 2) * HPC
        od = res.results[c]["OD"].reshape(HPC, D + 1, S)
        out[b, h0:h0 + HPC] = (od[:, :D, :] /
                               od[:, D:D + 1, :]).transpose(0, 2, 1)
    return out
